# revision 18
# baseline (speedup 1.0000x reference)
"""Trainium2 Bass kernel for nn_Detector (YOLO-style decode + per-image NMS).

Strategy (8 NeuronCores, data-parallel over batch):
  Kernel 1 (per core, 8 images): stream the objectness-confidence channels,
    per-partition top-k extraction (max8/max_index/match_replace rounds) on
    raw logits -> candidate (value, index) lists. 64x reduction on device.
  Host: merge per-core candidates -> exact global top-512 per scale
    (bit-exact jax/CPU sigmoid ordering), gather the 85-vectors of winners,
    global score sort, assign boxes to their owning core (per-image NMS).
  Kernel 2 (per core, <=256 boxes): decode (sigmoid/exp/argmax), pairwise
    IoU + same-image/same-class suppression matrix, greedy NMS solved by
    Jacobi fixpoint iteration on the sorted order (+1 extra iteration so the
    host can verify the fixpoint; exact-greedy fallback on host if not).
"""
import os
import sys

for _p in ("/opt/trn_rl_repo", os.path.expanduser("~/.axon_site/_ro/trn_rl_repo")):
    if os.path.isdir(_p) and _p not in sys.path:
        sys.path.append(_p)

import numpy as np

import concourse.bacc as bacc
import concourse.bass as bass
import concourse.mybir as mybir
import concourse.tile as tile
from concourse import bass_utils
from concourse import masks

F32 = mybir.dt.float32
U32 = mybir.dt.uint32
AF = mybir.ActivationFunctionType
ALU = mybir.AluOpType

N_CORES = 8
B_TOTAL = 64
B_PER = 8            # images per core
K = 512              # top-k per scale (global)
NBOX = 3 * K
M = 256              # per-core NMS capacity (padded); ~192 expected
T_JACOBI = 2         # device Jacobi iterations (real data converges in 2)
NEG = -1e30

# (name, H, stride, P_layout, F_layout, rounds)
# rounds*8 = per-partition candidate capacity; global winners per partition
# measured at <=4, and the host verifies completeness (falls back if not).
SCALES = [
    ("13", 13, 32.0, 104, 39, 3),
    ("26", 26, 16.0, 96, 169, 3),
    ("52", 52, 8.0, 128, 507, 3),
]

_cache = {}
PROFILE = False       # set True (e.g. from test.py) to capture NTFF exec times
LAST_EXEC_NS = {}
LAST_FALLBACKS = 0    # cores where host NMS fallback was used on the last call


def _cpu_sigmoid(x: np.ndarray) -> np.ndarray:
    """Bit-exact match of the reference's jax.nn.sigmoid on CPU."""
    import jax
    import jax.numpy as jnp

    dev = _cache.get("cpu_dev")
    if dev is None:
        dev = jax.devices("cpu")[0]
        _cache["cpu_dev"] = dev
    with jax.default_device(dev):
        return np.asarray(jax.nn.sigmoid(jnp.asarray(x, dtype=jnp.float32)))


# --------------------------------------------------------------------------
# Kernel 1: confidence-channel top-k candidates per scale
# --------------------------------------------------------------------------
def _build_k1():
    nc = bacc.Bacc("TRN2", target_bir_lowering=False, debug=False,
                   enable_asserts=False, num_devices=N_CORES)
    ins, outs = {}, {}
    for (s, H, st, P, F, R) in SCALES:
        ins[s] = nc.dram_tensor(f"conf{s}", [P, F], F32, kind="ExternalInput").ap()
        outs[s] = (
            nc.dram_tensor(f"vals{s}", [P, 8 * R], F32, kind="ExternalOutput").ap(),
            nc.dram_tensor(f"idx{s}", [P, 8 * R], U32, kind="ExternalOutput").ap(),
        )
    with tile.TileContext(nc) as tc:
        with tc.tile_pool(name="pool", bufs=1) as pool:
            for (s, H, st, P, F, R) in SCALES:
                grid = pool.tile([P, F], F32, name=f"grid{s}")
                vals = pool.tile([P, 8 * R], F32, name=f"vals{s}_t")
                idx = pool.tile([P, 8 * R], U32, name=f"idx{s}_t")
                nc.sync.dma_start(grid[:], ins[s])
                for r in range(R):
                    vsl = vals[:, r * 8:(r + 1) * 8]
                    isl = idx[:, r * 8:(r + 1) * 8]
                    nc.vector.max(vsl, grid[:])
                    nc.vector.max_index(isl, vsl, grid[:])
                    if r < R - 1:
                        nc.vector.match_replace(grid[:], vsl, grid[:], NEG)
                nc.sync.dma_start(outs[s][0], vals[:])
                nc.sync.dma_start(outs[s][1], idx[:])
    nc.compile()
    return nc


# --------------------------------------------------------------------------
# Kernel 2: decode + per-image NMS over the core's sorted boxes
# --------------------------------------------------------------------------
def _build_k2():
    nc = bacc.Bacc("TRN2", target_bir_lowering=False, debug=False,
                   enable_asserts=False, num_devices=N_CORES)
    NT = M // 128  # i-tiles

    v_in = nc.dram_tensor("v", [M, 85], F32, kind="ExternalInput").ap()
    # meta cols: 0=n 1=w_raw 2=h_raw 3=stride 4=anchor_w 5=anchor_h
    meta_in = nc.dram_tensor("meta", [M, 6], F32, kind="ExternalInput").ap()
    val_in = nc.dram_tensor("valrow", [1, M], F32, kind="ExternalInput").ap()
    boxes_out = nc.dram_tensor("boxes", [M, 7], F32, kind="ExternalOutput").ap()
    kT_out = nc.dram_tensor("kT", [1, M], F32, kind="ExternalOutput").ap()
    kT1_out = nc.dram_tensor("kT1", [1, M], F32, kind="ExternalOutput").ap()

    with tile.TileContext(nc) as tc:
        with (
            tc.tile_pool(name="pool", bufs=1) as pool,
            tc.tile_pool(name="psum", bufs=2, space="PSUM") as psum,
        ):
            ident = pool.tile([128, 128], F32)
            masks.make_identity(nc, ident[:])
            ones1 = pool.tile([1, 128], F32)
            nc.vector.memset(ones1[:], 1.0)

            valrow = pool.tile([1, M], F32)
            nc.sync.dma_start(valrow[:], val_in)

            # load both v/meta tiles first, then group activations by function
            # so the scalar engine loads each table (Sigmoid, Exp) only once
            vts, mts, sig3s, exp2s = [], [], [], []
            for t in range(NT):
                sl = slice(t * 128, (t + 1) * 128)
                vt = pool.tile([128, 85], F32, name=f"v{t}")
                mt = pool.tile([128, 6], F32, name=f"m{t}")
                nc.sync.dma_start(vt[:], v_in[sl, :])
                nc.sync.dma_start(mt[:], meta_in[sl, :])
                vts.append(vt); mts.append(mt)
                sig3s.append(pool.tile([128, 3], F32, name=f"sig3_{t}"))
                exp2s.append(pool.tile([128, 2], F32, name=f"exp2_{t}"))
            for t in range(NT):
                nc.scalar.activation(sig3s[t][:], vts[t][:, 0:3], AF.Sigmoid)
            for t in range(NT):
                nc.scalar.activation(exp2s[t][:], vts[t][:, 3:5], AF.Exp)

            # QrowT accumulates the PE-transposed per-box quantities
            # rows: 0=x1 1=y1 2=x2 3=y2 4=area 5=combo(n*128+cls)
            QrowT = pool.tile([8, M], F32)
            Qs = []
            for t in range(NT):
                sl = slice(t * 128, (t + 1) * 128)
                vt = vts[t]; mt = mts[t]
                sig3 = sig3s[t]; exp2 = exp2s[t]
                box = pool.tile([128, 7], F32, name=f"box{t}")
                # n, cond
                nc.vector.tensor_copy(box[:, 0:1], mt[:, 0:1])
                nc.vector.tensor_copy(box[:, 1:2], sig3[:, 0:1])
                # cx = (w + sig(v1)) * stride ; cy = (h + sig(v2)) * stride
                nc.vector.tensor_add(box[:, 2:3], mt[:, 1:2], sig3[:, 1:2])
                nc.vector.tensor_mul(box[:, 2:3], box[:, 2:3], mt[:, 3:4])
                nc.vector.tensor_add(box[:, 3:4], mt[:, 2:3], sig3[:, 2:3])
                nc.vector.tensor_mul(box[:, 3:4], box[:, 3:4], mt[:, 3:4])
                # bw = aw * exp(v3) ; bh = ah * exp(v4)
                nc.vector.tensor_mul(box[:, 4:5], mt[:, 4:5], exp2[:, 0:1])
                nc.vector.tensor_mul(box[:, 5:6], mt[:, 5:6], exp2[:, 1:2])
                # cls = argmax(v[5:85])
                cmax = pool.tile([128, 8], F32, name=f"cmax{t}")
                cidx = pool.tile([128, 8], U32, name=f"cidx{t}")
                nc.vector.max(cmax[:], vt[:, 5:85])
                nc.vector.max_index(cidx[:], cmax[:], vt[:, 5:85])
                nc.vector.tensor_copy(box[:, 6:7], cidx[:, 0:1])

                nc.sync.dma_start(boxes_out[sl, :], box[:])

                # Q = [x1 y1 x2 y2 area n cls val]
                Q = pool.tile([128, 8], F32, name=f"Q{t}")
                half = pool.tile([128, 2], F32, name=f"half{t}")
                nc.vector.tensor_scalar_mul(half[:, 0:1], box[:, 4:5], 0.5)
                nc.vector.tensor_scalar_mul(half[:, 1:2], box[:, 5:6], 0.5)
                nc.vector.tensor_sub(Q[:, 0:1], box[:, 2:3], half[:, 0:1])
                nc.vector.tensor_sub(Q[:, 1:2], box[:, 3:4], half[:, 1:2])
                nc.vector.tensor_add(Q[:, 2:3], box[:, 2:3], half[:, 0:1])
                nc.vector.tensor_add(Q[:, 3:4], box[:, 3:4], half[:, 1:2])
                nc.vector.tensor_mul(Q[:, 4:5], box[:, 4:5], box[:, 5:6])
                # combo = n*128 + cls (unique per (image, class), exact in f32)
                nc.vector.tensor_scalar_mul(Q[:, 5:6], box[:, 0:1], 128.0)
                nc.vector.tensor_add(Q[:, 5:6], Q[:, 5:6], box[:, 6:7])
                Qs.append(Q)

                # one PE transpose per tile: [128,8] -> [8,128]
                qT = psum.tile([8, 128], F32, name=f"qT{t}", tag="qT")
                nc.tensor.transpose(qT[:], Q[:], ident[:])
                nc.vector.tensor_copy(QrowT[:, t * 128:(t + 1) * 128], qT[:])

            # gather the 6 quantity rows onto partition 0 (DMA has no
            # partition-base restriction), then 3 psum-bank-wide broadcasts
            Qflat = pool.tile([1, 6 * M], F32)
            for q in range(6):
                nc.sync.dma_start(Qflat[:, q * M:(q + 1) * M], QrowT[q:q + 1, :])
            Ball = pool.tile([128, 6 * M], F32)
            for k in range(3):
                pb = psum.tile([128, 512], F32, name=f"pb{k}", tag="pb")
                nc.tensor.matmul(pb[:], ones1[:], Qflat[:, k * 512:(k + 1) * 512])
                nc.vector.tensor_copy(Ball[:, k * 512:(k + 1) * 512], pb[:])
            Bq = [Ball[:, q * M:(q + 1) * M] for q in range(6)]

            # suppression matrix O_t[i_local, j] (strict upper triangle).
            # For tile t, columns j < t*128 are identically zero: compute only
            # the live j-range and memset the rest once.
            Os = []
            for t in range(NT):
                Q = Qs[t]
                j0 = t * 128
                W = M - j0
                Ot = pool.tile([128, M], F32, name=f"O{t}")
                if j0:
                    nc.vector.memset(Ot[:, 0:j0], 0.0)
                O = Ot[:, j0:M]
                B = [Bq[q][:, j0:M] for q in range(6)]
                w1 = pool.tile([128, W], F32, name=f"w1_{t}")
                w2 = pool.tile([128, W], F32, name=f"w2_{t}")
                w3 = pool.tile([128, W], F32, name=f"w3_{t}")
                # iw = relu(min(x2i,x2j) - max(x1i,x1j))
                nc.vector.tensor_scalar_max(w1[:], B[0], Q[:, 0:1])
                nc.vector.tensor_scalar_min(w2[:], B[2], Q[:, 2:3])
                nc.vector.tensor_sub(w1[:], w2[:], w1[:])
                nc.vector.tensor_scalar_max(w1[:], w1[:], 0.0)
                # ih
                nc.vector.tensor_scalar_max(w3[:], B[1], Q[:, 1:2])
                nc.vector.tensor_scalar_min(w2[:], B[3], Q[:, 3:4])
                nc.vector.tensor_sub(w2[:], w2[:], w3[:])
                nc.vector.tensor_scalar_max(w2[:], w2[:], 0.0)
                # inter
                nc.vector.tensor_mul(w1[:], w1[:], w2[:])
                # union = areai + areaj - inter
                nc.vector.tensor_scalar_add(w2[:], B[4], Q[:, 4:5])
                nc.vector.tensor_sub(w2[:], w2[:], w1[:])
                # iou > 0.1  <=>  10*inter > union
                nc.vector.tensor_scalar_mul(w1[:], w1[:], 10.0)
                nc.vector.tensor_tensor(O, w1[:], w2[:], ALU.is_gt)
                # same (image, class) via the packed combo key
                nc.vector.tensor_scalar(w1[:], B[5], Q[:, 5:6], None, ALU.is_equal)
                nc.vector.tensor_mul(O, O, w1[:])
                # strict upper triangle: keep jj > p (jj relative to j0)
                nc.gpsimd.affine_select(
                    out=O, in_=O, pattern=[[1, W]],
                    compare_op=ALU.is_ge, fill=0.0,
                    base=-1, channel_multiplier=-1,
                )
                Os.append(Ot)

            # Jacobi fixpoint: g <- val * (O^T g < 0.5)
            grow = pool.tile([1, M], F32)
            nc.vector.tensor_copy(grow[:], valrow[:])
            for it in range(T_JACOBI + 1):
                gcol = pool.tile([128, NT], F32, name=f"gcol{it}")
                for t in range(NT):
                    nc.sync.dma_start(gcol[:, t:t + 1],
                                      grow[:, t * 128:(t + 1) * 128])
                ps = psum.tile([1, M], F32, name=f"ps{it}", tag="ps")
                for t in range(NT):
                    nc.tensor.matmul(ps[:], gcol[:, t:t + 1], Os[t][:],
                                     start=(t == 0), stop=(t == NT - 1))
                if it == T_JACOBI:
                    nc.sync.dma_start(kT_out, grow[:])
                cmp = pool.tile([1, M], F32, name=f"cmp{it}")
                nc.vector.tensor_single_scalar(cmp[:], ps[:], 0.5, ALU.is_lt)
                nc.vector.tensor_tensor(grow[:], cmp[:], valrow[:], ALU.mult)
            nc.sync.dma_start(kT1_out, grow[:])
    nc.compile()
    return nc


def _get(name):
    nc = _cache.get(name)
    if nc is None:
        nc = {"k1": _build_k1, "k2": _build_k2}[name]()
        _cache[name] = nc
    return nc


# --------------------------------------------------------------------------
# Host-side exact-greedy fallback NMS (used if Jacobi did not reach fixpoint
# or a core holds more than M boxes). Mirrors the device math in fp32.
# --------------------------------------------------------------------------
def _host_core_nms(v, meta, sig):
    m = v.shape[0]
    with np.errstate(over="ignore"):
        sx = 1.0 / (1.0 + np.exp(-v[:, 1]))
        sy = 1.0 / (1.0 + np.exp(-v[:, 2]))
        cond = 1.0 / (1.0 + np.exp(-v[:, 0]))
    ew = np.exp(v[:, 3]); eh = np.exp(v[:, 4])
    n = meta[:, 0]
    cx = ((meta[:, 1] + sx) * meta[:, 3]).astype(np.float32)
    cy = ((meta[:, 2] + sy) * meta[:, 3]).astype(np.float32)
    bw = (meta[:, 4] * ew).astype(np.float32)
    bh = (meta[:, 5] * eh).astype(np.float32)
    cls = np.argmax(v[:, 5:85], axis=1).astype(np.float32)
    cond = sig  # use the bit-exact sigmoid for the reported confidence
    boxes7 = np.stack([n, cond, cx, cy, bw, bh, cls], axis=1).astype(np.float32)
    val = v[:, 0] > 0.0

    x1 = cx - bw * 0.5; y1 = cy - bh * 0.5
    x2 = cx + bw * 0.5; y2 = cy + bh * 0.5
    area = bw * bh
    ix1 = np.maximum(x1[:, None], x1[None, :])
    iy1 = np.maximum(y1[:, None], y1[None, :])
    ix2 = np.minimum(x2[:, None], x2[None, :])
    iy2 = np.minimum(y2[:, None], y2[None, :])
    inter = np.clip(ix2 - ix1, 0, None) * np.clip(iy2 - iy1, 0, None)
    union = area[:, None] + area[None, :] - inter
    ov = (inter * np.float32(10.0) > union)
    ov &= (n[:, None] == n[None, :]) & (cls[:, None] == cls[None, :])

    keep = np.zeros(m, bool)
    supp = np.zeros(m, bool)
    for i in range(m):
        if val[i] and not supp[i]:
            keep[i] = True
            supp |= ov[i] & (np.arange(m) > i)
    return boxes7, keep


# --------------------------------------------------------------------------
# The public entry point
# --------------------------------------------------------------------------
def kernel(output_13, output_26, output_52, anchors_13, anchors_26, anchors_52):
    outs = {"13": np.ascontiguousarray(output_13, dtype=np.float32),
            "26": np.ascontiguousarray(output_26, dtype=np.float32),
            "52": np.ascontiguousarray(output_52, dtype=np.float32)}
    anch = {"13": np.asarray(anchors_13, dtype=np.float32),
            "26": np.asarray(anchors_26, dtype=np.float32),
            "52": np.asarray(anchors_52, dtype=np.float32)}

    # ---- kernel 1: per-core confidence top-k candidates --------------------
    nc1 = _get("k1")
    in_maps = []
    conf_store = {s: [] for (s, *_r) in SCALES}
    for c in range(N_CORES):
        im = {}
        for (s, H, st, P, F, R) in SCALES:
            HW = H * H
            shard = outs[s][c * B_PER:(c + 1) * B_PER]
            conf = np.ascontiguousarray(
                shard[:, 0:255:85, :, :]).reshape(P, F)
            im[f"conf{s}"] = conf
            conf_store[s].append(conf)
        in_maps.append(im)
    res1 = bass_utils.run_bass_kernel_spmd(nc1, in_maps, core_ids=list(range(N_CORES)),
                                           trace=PROFILE)
    if PROFILE:
        LAST_EXEC_NS["k1"] = res1.exec_time_ns

    # ---- host merge: exact global top-512 per scale ------------------------
    per_scale = []
    for (s, H, st, P, F, R) in SCALES:
        HW = H * H
        ridx, rlog = [], []
        for c in range(N_CORES):
            vals = res1.results[c][f"vals{s}"].reshape(-1)
            idx = res1.results[c][f"idx{s}"].astype(np.int64).reshape(P, 8 * R)
            q = (np.arange(P)[:, None] * F + idx).reshape(-1)
            bl = q // (3 * HW); r0 = q % (3 * HW)
            a = r0 // HW; hw = r0 % HW
            b = c * B_PER + bl
            ridx.append((b * HW + hw) * 3 + a)
            rlog.append(vals)
        ridx = np.concatenate(ridx); rlog = np.concatenate(rlog)
        sig = _cpu_sigmoid(rlog)
        order = np.lexsort((ridx, -sig))[:K]
        refidx = ridx[order]; sigk = sig[order]; logk = rlog[order]

        # completeness: every partition's minimum extracted value must rank
        # strictly below the 512th selected score, else winners could hide
        # beyond the per-partition capacity -> redo top-k on host exactly.
        minlog = np.stack([res1.results[c][f"vals{s}"][:, -1]
                           for c in range(N_CORES)])
        if not np.all(_cpu_sigmoid(minlog) < sigk[-1]):
            full_log, full_idx = [], []
            for c in range(N_CORES):
                q = np.arange(P * F, dtype=np.int64)
                bl = q // (3 * HW); r0 = q % (3 * HW)
                a = r0 // HW; hw = r0 % HW
                b = c * B_PER + bl
                full_idx.append((b * HW + hw) * 3 + a)
                full_log.append(conf_store[s][c].reshape(-1))
            full_idx = np.concatenate(full_idx)
            full_sig = _cpu_sigmoid(np.concatenate(full_log))
            order = np.lexsort((full_idx, -full_sig))[:K]
            refidx = full_idx[order]; sigk = full_sig[order]
            logk = np.concatenate(full_log)[order]

        b, h, w, a = np.unravel_index(refidx, (B_TOTAL, H, H, 3))
        o = outs[s]
        v = o[b[:, None], a[:, None] * 85 + np.arange(85)[None, :],
              h[:, None], w[:, None]].astype(np.float32)
        meta = np.stack([
            b.astype(np.float32), w.astype(np.float32), h.astype(np.float32),
            np.full(K, st, np.float32),
            anch[s][a, 0], anch[s][a, 1],
        ], axis=1).astype(np.float32)
        per_scale.append((v, meta, sigk, logk))

    v_all = np.concatenate([p[0] for p in per_scale])
    meta_all = np.concatenate([p[1] for p in per_scale])
    sig_all = np.concatenate([p[2] for p in per_scale])
    log_all = np.concatenate([p[3] for p in per_scale])

    valid = log_all > 0.0
    score = np.where(valid, sig_all, -np.inf).astype(np.float32)
    order = np.argsort(-score, kind="stable")
    v_s = v_all[order]; meta_s = meta_all[order]
    sig_s = sig_all[order]; val_s = valid[order]
    img_s = meta_s[:, 0].astype(np.int64)
    core_s = img_s // B_PER

    # ---- kernel 2: per-core decode + NMS -----------------------------------
    nc2 = _get("k2")
    core_rows = [np.nonzero(core_s == c)[0] for c in range(N_CORES)]
    in_maps2 = []
    for c in range(N_CORES):
        rows = core_rows[c]
        m = min(len(rows), M)
        v_in = np.zeros((M, 85), np.float32)
        meta_in = np.zeros((M, 6), np.float32)
        val_in = np.zeros((1, M), np.float32)
        v_in[:m] = v_s[rows[:m]]
        meta_in[:m] = meta_s[rows[:m]]
        val_in[0, :m] = val_s[rows[:m]]
        in_maps2.append({"v": v_in, "meta": meta_in, "valrow": val_in})
    res2 = bass_utils.run_bass_kernel_spmd(nc2, in_maps2, core_ids=list(range(N_CORES)),
                                           trace=PROFILE)
    if PROFILE:
        LAST_EXEC_NS["k2"] = res2.exec_time_ns

    # ---- assemble ----------------------------------------------------------
    global LAST_FALLBACKS
    LAST_FALLBACKS = 0
    final_boxes = np.zeros((NBOX, 7), np.float32)
    final_keep = np.zeros(NBOX, bool)
    for c in range(N_CORES):
        rows = core_rows[c]
        m = len(rows)
        r = res2.results[c]
        kT = r["kT"][0, :m] if m <= M else None
        kT1 = r["kT1"][0, :m] if m <= M else None
        if m <= M and np.array_equal(kT, kT1):
            keep = kT > 0.5
            boxes7 = r["boxes"][:m]
        else:
            LAST_FALLBACKS += 1
            boxes7, keep = _host_core_nms(v_s[rows], meta_s[rows], sig_s[rows])
        final_boxes[rows] = boxes7 * keep[:, None].astype(np.float32)
        final_keep[rows] = keep
    return final_boxes, final_keep
